# revision 1
# baseline (speedup 1.0000x reference)
"""BPNet GNN message-passing kernel for 8 Trainium2 NeuronCores.

Strategy (forced by this image: no extended-GPSIMD ucode, no indirect DMA —
both crash the device, verified experimentally; only static DMA + PE + DVE/ACT
work):
  - Node-sharded output: core c owns nodes [512c, 512c+512). Every (edge,slot)
    pair is routed (host-side marshaling) to the owner core of its target node.
  - Host packs, per core, a position stream of pairs grouped by
    (node-chunk q in [0,4), slot i in [0,3)) -> 12 groups, each padded to whole
    128-position tiles. All per-pair data is baked into dense device tensors:
      xab  [128, L]  bf16 : one-hot-placed gathered node features + type
                            indicator for the pair's two COMPANION slots
                            (K-dim one-hot folds the per-edge weight selection
                            and bias into one fixed matmul).
      msel [128,T*64] bf16 : per-pair output-type selection mask (4x16 blocks)
      bsel [128,T*16] bf16 : per-pair pre-selected ho_bias row
      oneh [128,T*128]bf16 : per-tile one-hot incidence (lane -> node row)
  - Device: stage1 t = relu(W1^T @ xab) twice (companion a, b); fact = ta*tb;
    stage2 per tile: psum[128,64] = fact_tile^T @ W2cat(slot); select =
    (psum * msel) block-summed + bsel; segment-sum per tile:
    psum_nodes[:, 16q:+16] += oneh_tile^T @ msg_tile (PSUM accumulation).
  - Output per core: [128, 64] f32 = nodes [row, 16q+d]; unshard = pure
    reshape/concat on host. All arithmetic on the device.
"""

import numpy as np
import ml_dtypes

N, E, ORDER, D, RANK = 4096, 16384, 3, 13, 128
NP_ = ORDER + 1  # 4 types
NCORES = 8
NODES_PER_CORE = N // NCORES  # 512

bf16 = ml_dtypes.bfloat16

_COMPILED = {}  # Tg -> (nc, names)


def _build_program(Tg: int):
    import concourse.bacc as bacc
    import concourse.tile as tile
    from concourse import mybir

    T = 12 * Tg
    L = 128 * T
    NCH = L // 512

    nc = bacc.Bacc("TRN2", target_bir_lowering=False, debug=False,
                   num_devices=NCORES)
    BF, F32 = mybir.dt.bfloat16, mybir.dt.float32
    Relu = mybir.ActivationFunctionType.Relu
    Copy = mybir.ActivationFunctionType.Copy
    mult = mybir.AluOpType.mult

    xabs = [nc.dram_tensor(f"xab{j}", [128, L // 3], BF, kind="ExternalInput").ap() for j in range(3)]
    w1a = nc.dram_tensor("w1a", [128, 128], BF, kind="ExternalInput").ap()
    w1b = nc.dram_tensor("w1b", [128, 128], BF, kind="ExternalInput").ap()
    w2 = nc.dram_tensor("w2", [128, 3, 64], BF, kind="ExternalInput").ap()
    msel = nc.dram_tensor("msel", [128, T // 4, 4, 64], BF, kind="ExternalInput").ap()
    bsel = nc.dram_tensor("bsel", [128, T // 4, 4, 16], BF, kind="ExternalInput").ap()
    onehs = [nc.dram_tensor(f"oneh{j}", [128, T * 64], BF, kind="ExternalInput").ap() for j in range(2)]
    out = nc.dram_tensor("out", [128, 64], F32, kind="ExternalOutput").ap()

    with tile.TileContext(nc) as tc:
        with tc.tile_pool(name="inp", bufs=1) as inp, \
             tc.tile_pool(name="work", bufs=1) as work, \
             tc.tile_pool(name="ps1", bufs=3, space="PSUM") as ps1, \
             tc.tile_pool(name="ps2", bufs=3, space="PSUM") as ps2, \
             tc.tile_pool(name="psn", bufs=1, space="PSUM") as psn:
            xab_sbs = [inp.tile([128, L // 3], BF, tag=f"xab{j}", name=f"xab{j}_sb") for j in range(3)]
            w1a_sb = inp.tile([128, 128], BF, tag="w1a")
            w1b_sb = inp.tile([128, 128], BF, tag="w1b")
            w2_sb = inp.tile([128, 3, 64], BF, tag="w2")
            msel_sb = inp.tile([128, T // 4, 4, 64], BF, tag="msel")
            bsel_sb = inp.tile([128, T // 4, 4, 16], BF, tag="bsel")
            oneh_sbs = [inp.tile([128, T * 64], BF, tag=f"oneh{j}", name=f"oneh{j}_sb") for j in range(2)]

            for j in range(3):
                nc.sync.dma_start(xab_sbs[j][:], xabs[j][:])
            nc.sync.dma_start(w1a_sb[:], w1a[:])
            nc.sync.dma_start(w1b_sb[:], w1b[:])
            nc.sync.dma_start(w2_sb[:, :, :], w2[:, :, :])
            nc.sync.dma_start(msel_sb[:, :, :, :], msel[:, :, :, :])
            nc.sync.dma_start(bsel_sb[:, :, :, :], bsel[:, :, :, :])
            for j in range(2):
                nc.sync.dma_start(oneh_sbs[j][:], onehs[j][:])

            ta = work.tile([128, L], BF, tag="ta")
            tb = work.tile([128, L], BF, tag="tb")
            fact = work.tile([128, L], BF, tag="fact")
            msg = work.tile([128, T // 4, 4, 16], BF, tag="msg")
            sel = work.tile([128, 4, 64], BF, tag="sel")
            f1 = work.tile([128, 4, 32], BF, tag="f1")
            f2 = work.tile([128, 4, 16], BF, tag="f2")
            oc = work.tile([128, 64], F32, tag="oc")

            # stage 1: t = relu(W1^T @ xab), companions a and b
            CH3 = NCH // 3  # chunks per xab third
            for w_sb, tdst in ((w1a_sb, ta), (w1b_sb, tb)):
                for ch in range(NCH):
                    p = ps1.tile([128, 512], F32, tag="p1")
                    sl = slice(512 * ch, 512 * (ch + 1))
                    lsl = slice(512 * (ch % CH3), 512 * (ch % CH3 + 1))
                    nc.tensor.matmul(p[:], w_sb[:], xab_sbs[ch // CH3][:, lsl],
                                     start=True, stop=True)
                    if ch % 2 == 0:
                        nc.vector.tensor_scalar_max(tdst[:, sl], p[:], 0.0)
                    else:
                        nc.scalar.activation(tdst[:, sl], p[:], Relu)

            # fact = ta * tb  (L is always a multiple of 1536)
            for ch in range(L // 1536):
                sl = slice(1536 * ch, 1536 * (ch + 1))
                nc.vector.tensor_mul(fact[:, sl], ta[:, sl], tb[:, sl])

            # stage 2 + select, superblocks of 4 tiles.
            # ScalarE copies PSUM->SBUF bf16; DVE select chain runs bf16 2x.
            for s in range(T // 4):
                pb = ps2.tile([128, 4, 64], F32, tag="p2")
                for k in range(4):
                    t = 4 * s + k
                    i_slot = (t // Tg) % 3
                    nc.tensor.matmul(
                        pb[:, k, :],
                        fact[:, 128 * t:128 * (t + 1)],
                        w2_sb[:, i_slot, :],
                        start=True, stop=True,
                    )
                pc = work.tile([128, 4, 64], BF, tag="pc")
                nc.scalar.activation(pc[:, :, :], pb[:, :, :], Copy)
                nc.vector.tensor_tensor(
                    sel[:, :, :], pc[:, :, :],
                    msel_sb[:, s, :, :], mult)
                nc.vector.tensor_add(f1[:, :, :], sel[:, :, 0:32],
                                     sel[:, :, 32:64])
                nc.vector.tensor_add(f2[:, :, :], f1[:, :, 0:16],
                                     f1[:, :, 16:32])
                nc.vector.tensor_add(msg[:, s, :, :], f2[:, :, :],
                                     bsel_sb[:, s, :, :])

            # segment sum: psum_nodes[:, 16q:+16] += oneh_t^T @ msg_t
            pn = psn.tile([128, 64], F32, tag="pn")
            for t in range(T):
                q = t // (3 * Tg)
                first = (t % (3 * Tg)) == 0
                last = (t % (3 * Tg)) == 3 * Tg - 1
                nc.tensor.matmul(
                    pn[:, 16 * q:16 * (q + 1)],
                    oneh_sbs[t // (T // 2)][:, 128 * (t % (T // 2)):
                                            128 * (t % (T // 2) + 1)],
                    msg[:, t // 4, t % 4, :],
                    start=first, stop=last,
                )

            nc.vector.tensor_copy(oc[:], pn[:])
            nc.sync.dma_start(out[:], oc[:])

    nc.compile()
    return nc


def _prep_inputs(nodes, bp_params, bp_bias, ho_params, ho_bias, edges,
                 edge_types):
    nodes = np.asarray(nodes, np.float32)
    bp_params = np.asarray(bp_params, np.float32)
    bp_bias = np.asarray(bp_bias, np.float32)
    ho_params = np.asarray(ho_params, np.float32)
    ho_bias = np.asarray(ho_bias, np.float32)
    edges = np.asarray(edges, np.int64)
    edge_types = np.asarray(edge_types, np.int64)

    nodes_b = nodes.astype(bf16)

    # group pairs: (core, q, i) ; pair list per group
    tgt = edges  # [E, 3]
    owner = tgt // NODES_PER_CORE
    q = (tgt % NODES_PER_CORE) // 128
    r = tgt % 128

    group_lists = {}
    maxcount = 1
    for c in range(NCORES):
        for i in range(ORDER):
            sel_c = owner[:, i] == c
            for qq in range(4):
                es = np.nonzero(sel_c & (q[:, i] == qq))[0]
                group_lists[(c, qq, i)] = es
                maxcount = max(maxcount, len(es))
    Tg = (maxcount + 127) // 128
    T = 12 * Tg
    L = 128 * T

    # weight tables (shared across cores)
    w1a = np.zeros((128, 128), np.float32)
    w1b = np.zeros((128, 128), np.float32)
    for p in range(NP_):
        w1a[13 * p:13 * p + 13, :] = bp_params[p]
        w1a[52 + p, :] = bp_bias[p, 0, :]
        w1b[64 + 13 * p:64 + 13 * p + 13, :] = bp_params[p]
        w1b[116 + p, :] = bp_bias[p, 0, :]
    w2 = np.zeros((128, 3, 64), np.float32)
    for i in range(ORDER):
        for p in range(NP_):
            w2[:, i, 16 * p:16 * p + 13] = ho_params[i, p]

    in_maps = []
    for c in range(NCORES):
        xab = np.zeros((128, L), np.float32)
        msel = np.zeros((128, T * 64), np.float32)
        bsel_a = np.zeros((128, T * 16), np.float32)
        oneh = np.zeros((128, T * 128), np.float32)
        for qq in range(4):
            for i in range(ORDER):
                es = group_lists[(c, qq, i)]
                g = qq * 3 + i
                k = np.arange(len(es))
                x = 128 * g * Tg + k
                t_arr = x // 128
                lane = x % 128
                a, b = (i + 1) % 3, (i + 2) % 3
                ta_t = edge_types[es, a]
                tb_t = edge_types[es, b]
                fa = nodes_b[edges[es, a]].astype(np.float32)  # [m, 13]
                fb = nodes_b[edges[es, b]].astype(np.float32)
                for dd in range(D):
                    xab[13 * ta_t + dd, x] = fa[:, dd]
                    xab[64 + 13 * tb_t + dd, x] = fb[:, dd]
                xab[52 + ta_t, x] = 1.0
                xab[116 + tb_t, x] = 1.0
                p_e = edge_types[es, i]
                for dd in range(D):
                    msel[lane, 64 * t_arr + 16 * p_e + dd] = 1.0
                    bsel_a[lane, 16 * t_arr + dd] = ho_bias[i, p_e, 0, dd]
                oneh[lane, 128 * t_arr + r[es, i]] = 1.0
        xab_b = xab.astype(bf16)
        oneh_b = oneh.astype(bf16)
        in_maps.append({
            "xab0": xab_b[:, 0:L // 3],
            "xab1": xab_b[:, L // 3:2 * L // 3],
            "xab2": xab_b[:, 2 * L // 3:],
            "w1a": w1a.astype(bf16),
            "w1b": w1b.astype(bf16),
            "w2": w2.astype(bf16),
            "msel": msel.reshape(128, T // 4, 4, 64).astype(bf16),
            "bsel": bsel_a.reshape(128, T // 4, 4, 16).astype(bf16),
            "oneh0": oneh_b[:, :T * 64],
            "oneh1": oneh_b[:, T * 64:],
        })
    return in_maps, Tg


def kernel(nodes, bp_params, bp_bias, ho_params, ho_bias, edges, edge_types,
           atoms=None, atom_edges=None, _run_kwargs=None):
    from concourse.bass_utils import run_bass_kernel_spmd

    in_maps, Tg = _prep_inputs(nodes, bp_params, bp_bias, ho_params, ho_bias,
                               edges, edge_types)
    if Tg not in _COMPILED:
        _COMPILED[Tg] = _build_program(Tg)
    nc = _COMPILED[Tg]

    res = run_bass_kernel_spmd(nc, in_maps, core_ids=list(range(NCORES)),
                               **(_run_kwargs or {}))
    full = np.zeros((N, D), np.float32)
    for c in range(NCORES):
        oc = res.results[c]["out"]  # [128, 64]
        for qq in range(4):
            full[NODES_PER_CORE * c + 128 * qq:
                 NODES_PER_CORE * c + 128 * (qq + 1), :] = oc[:, 16 * qq:
                                                             16 * qq + 13]
    kernel._last_result = res
    return full



# revision 4
# speedup vs baseline: 1.3212x; 1.3212x over previous
"""BPNet GNN message-passing kernel for 8 Trainium2 NeuronCores.

Strategy (no indirect DMA / no extended-GPSIMD ucode on this image; only
static DMA + PE + DVE/ACT are usable):
  - Node-sharded output: host assigns the 4096 nodes to 32 chunks of 128
    (core c owns 4 chunks = rows of its [128, 64] output). The assignment is
    LOAD-BALANCED (greedy vector bin-packing on per-slot node degrees) so the
    per-(core, chunk, slot) pair-group maxima are small -> fewer padded tiles.
  - Every (edge,slot) pair is routed to the owner core of its target node.
    Pairs are grouped by (chunk q in [0,4), slot i in [0,3)) -> 12 groups,
    group g gets a host-chosen tile count Tg[g] (ceil of the max count over
    cores / 128). All per-pair data is baked into dense device tensors:
      xab  [128, L]  bf16 : one-hot-placed gathered node features + type
                            indicator for the pair's two COMPANION slots.
      msb  [128,SB,4,5,16] : blocks 0..3 = per-pair output-type selection
                            mask; block 4 = pre-selected ho_bias row.
      oneh [128,SB,4,128] : per-tile one-hot incidence (lane -> node row).
  - Device, software-pipelined per superblock sb (4 tiles = 512 pairs):
      stage1: pa = w1a^T @ xab_sb, pb = w1b^T @ xab_sb (PSUM);
              tb = relu(pb) [ACT]; fact = max(pa,0)*tb [DVE fused STT].
      stage2: per tile: psum[128,64] = fact_t^T @ w2[slot]; pc = copy bf16
              [ACT]; sel = pc*msel [DVE]; red = sum over the 4 type blocks
              [DVE strided tensor_reduce]; msg = red + bsel [DVE].
      seg:    psum_nodes[:, 16q:+16] += oneh_t^T @ msg_t (PSUM accumulate).
  - All per-superblock input DMAs are issued from the (otherwise idle)
    GPSIMD sequencer in consumption order so compute starts ~1.5us in and
    loads stream concurrently with compute.
  - Output per core: [128, 64] f32; unshard on host via the node map.
"""

import numpy as np
import ml_dtypes

N, E, ORDER, D, RANK = 4096, 16384, 3, 13, 128
NP_ = ORDER + 1  # 4 types
NCORES = 8
NODES_PER_CORE = N // NCORES  # 512
NCHUNKS = 32  # 32 chunks of 128 nodes

bf16 = ml_dtypes.bfloat16

_COMPILED = {}  # tuple(Tg) -> nc


def _tile_map(Tgs):
    """tile t -> (q, i); per-q first/last tile index."""
    tq, ti = [], []
    for g, tg in enumerate(Tgs):
        q, i = g // 3, g % 3
        tq += [q] * tg
        ti += [i] * tg
    qstart = {}
    qend = {}
    for t, q in enumerate(tq):
        if q not in qstart:
            qstart[q] = t
        qend[q] = t
    return tq, ti, qstart, qend


def _build_program(Tgs):
    import concourse.bacc as bacc
    import concourse.tile as tile
    from concourse import mybir

    T = sum(Tgs)
    SB = T // 4
    L = 128 * T
    tq, ti, qstart, qend = _tile_map(Tgs)

    nc = bacc.Bacc("TRN2", target_bir_lowering=False, debug=False,
                   num_devices=NCORES)
    BF, F32 = mybir.dt.bfloat16, mybir.dt.float32
    Relu = mybir.ActivationFunctionType.Relu
    Copy = mybir.ActivationFunctionType.Copy
    mult = mybir.AluOpType.mult
    amax = mybir.AluOpType.max
    aadd = mybir.AluOpType.add

    xab = nc.dram_tensor("xab", [128, L], BF, kind="ExternalInput").ap()
    w1a = nc.dram_tensor("w1a", [128, 128], BF, kind="ExternalInput").ap()
    w1b = nc.dram_tensor("w1b", [128, 128], BF, kind="ExternalInput").ap()
    w2 = nc.dram_tensor("w2", [128, 3, 64], BF, kind="ExternalInput").ap()
    msb = nc.dram_tensor("msb", [128, SB, 4, 5, 16], BF, kind="ExternalInput").ap()
    oneh = nc.dram_tensor("oneh", [128, SB, 4, 128], BF, kind="ExternalInput").ap()
    out = nc.dram_tensor("out", [128, 64], F32, kind="ExternalOutput").ap()

    with tile.TileContext(nc) as tc:
        with tc.tile_pool(name="inp", bufs=1) as inp, \
             tc.tile_pool(name="work", bufs=1) as work, \
             tc.tile_pool(name="tbp", bufs=3) as tbp, \
             tc.tile_pool(name="factp", bufs=3) as factp, \
             tc.tile_pool(name="selp", bufs=2) as selp, \
             tc.tile_pool(name="pcp", bufs=2) as pcp, \
             tc.tile_pool(name="redp", bufs=2) as redp, \
             tc.tile_pool(name="msgp", bufs=2) as msgp, \
             tc.tile_pool(name="ps1", bufs=4, space="PSUM") as ps1, \
             tc.tile_pool(name="ps2", bufs=2, space="PSUM") as ps2, \
             tc.tile_pool(name="psn", bufs=1, space="PSUM") as psn:

            w1a_sb = inp.tile([128, 128], BF, tag="w1a")
            w1b_sb = inp.tile([128, 128], BF, tag="w1b")
            w2_sb = inp.tile([128, 3, 64], BF, tag="w2")
            xab_sbs = [inp.tile([128, 512], BF, tag=f"xab{s}",
                                name=f"xab{s}_sb") for s in range(SB)]
            msb_sbs = [inp.tile([128, 4, 5, 16], BF, tag=f"msb{s}",
                                name=f"msb{s}_sb") for s in range(SB)]
            oneh_sbs = [inp.tile([128, 4, 128], BF, tag=f"oneh{s}",
                                 name=f"oneh{s}_sb") for s in range(SB)]
            warm = work.tile([128, 1], F32, tag="warm")
            oc = work.tile([128, 64], F32, tag="oc")

            # Warm up the ACT table for Relu during the DMA lead-in.
            nc.gpsimd.memset(warm[:], 0.0)
            nc.scalar.activation(warm[:], warm[:], Relu)

            # Weights via SP; per-superblock streams via GPSIMD (cheap
            # 25ns/issue sequencer) in consumption order.
            nc.sync.dma_start(w1a_sb[:], w1a[:])
            nc.sync.dma_start(w1b_sb[:], w1b[:])
            nc.sync.dma_start(w2_sb[:, :, :], w2[:, :, :])
            for s in range(min(2, SB)):
                nc.gpsimd.dma_start(xab_sbs[s][:], xab[:, 512 * s:512 * (s + 1)])
            for s in range(SB):
                if s + 2 < SB:
                    nc.gpsimd.dma_start(xab_sbs[s + 2][:],
                                        xab[:, 512 * (s + 2):512 * (s + 3)])
                nc.gpsimd.dma_start(oneh_sbs[s][:, :, :], oneh[:, s, :, :])
                nc.gpsimd.dma_start(msb_sbs[s][:, :, :, :], msb[:, s, :, :, :])

            pn = psn.tile([128, 64], F32, tag="pn")

            def stage1(s):
                pa = ps1.tile([128, 512], F32, tag="p1", name=f"pa{s}")
                pb = ps1.tile([128, 512], F32, tag="p1", name=f"pb{s}")
                nc.tensor.matmul(pa[:], w1a_sb[:], xab_sbs[s][:],
                                 start=True, stop=True)
                nc.tensor.matmul(pb[:], w1b_sb[:], xab_sbs[s][:],
                                 start=True, stop=True)
                tb = tbp.tile([128, 512], BF, tag="tb", name=f"tb{s}")
                nc.scalar.activation(tb[:], pb[:], Relu)
                fact = factp.tile([128, 512], BF, tag="fact", name=f"fact{s}")
                nc.vector.scalar_tensor_tensor(
                    fact[:], pa[:], 0.0, tb[:], amax, mult)
                return fact

            def stage2_seg(s, fact):
                pb4 = ps2.tile([128, 4, 4, 16], F32, tag="p2", name=f"pb4_{s}")
                for k in range(4):
                    t = 4 * s + k
                    nc.tensor.matmul(pb4[:, k, :, :], fact[:, 128 * k:128 * (k + 1)],
                                     w2_sb[:, ti[t], :], start=True, stop=True)
                pc = pcp.tile([128, 4, 4, 16], BF, tag="pc", name=f"pc{s}")
                nc.scalar.activation(pc[:, :, :, :], pb4[:, :, :, :], Copy)
                sel = selp.tile([128, 4, 4, 16], BF, tag="sel", name=f"sel{s}")
                nc.vector.tensor_tensor(
                    sel[:, :, :, :], pc[:, :, :, :],
                    msb_sbs[s][:, :, 0:4, :], mult)
                red = redp.tile([128, 4, 16], F32, tag="red", name=f"red{s}")
                nc.vector.tensor_reduce(
                    red[:, :, :], sel[:, :, :, :].transpose([0, 1, 3, 2]),
                    mybir.AxisListType.X, aadd)
                msg = msgp.tile([128, 4, 16], BF, tag="msg", name=f"msg{s}")
                nc.vector.tensor_tensor(
                    msg[:, :, :], red[:, :, :], msb_sbs[s][:, :, 4, :], aadd)
                for k in range(4):
                    t = 4 * s + k
                    q = tq[t]
                    nc.tensor.matmul(
                        pn[:, 16 * q:16 * (q + 1)],
                        oneh_sbs[s][:, k, :], msg[:, k, :],
                        start=(t == qstart[q]), stop=(t == qend[q]),
                        skip_group_check=True)

            prev = None
            for s in range(SB):
                fact = stage1(s)
                if prev is not None:
                    stage2_seg(s - 1, prev)
                prev = fact
            stage2_seg(SB - 1, prev)

            nc.vector.tensor_copy(oc[:], pn[:])
            nc.sync.dma_start(out[:], oc[:])

    nc.compile()
    return nc


def _balance(edges):
    """Assign nodes to 32 chunks of <=128, balancing per-slot pair counts;
    then chunks -> (core, q) with the heaviest chunks concentrated in the
    same q so overflow tiles are shared. Returns (c_of, q_of, r_of, Tgs)."""
    deg = np.zeros((N, ORDER), np.int64)
    for i in range(ORDER):
        np.add.at(deg[:, i], edges[:, i], 1)
    order = np.argsort(-deg.sum(1), kind="stable")
    loads = np.zeros((NCHUNKS, ORDER), np.float64)
    counts = np.zeros(NCHUNKS, np.int64)
    chunk_of = np.empty(N, np.int64)
    for n in order:
        d = deg[n]
        cand = np.maximum(loads + d, 0).max(1) + 1e-3 * (loads + d).sum(1)
        cand[counts >= 128] = 1e18
        ch = int(np.argmin(cand))
        chunk_of[n] = ch
        loads[ch] += d
        counts[ch] += 1

    # Local search: per-slot totals are exactly 32*512, so the optimum is
    # every (chunk, slot) load == 512 (zero overflow -> 4 tiles per group).
    # Swap nodes between chunks while the swap reduces total overflow.
    def overflow(ld):
        return np.maximum(ld - 512.0, 0.0).sum()

    degf = deg.astype(np.float64)
    rng = np.random.default_rng(0)
    stall = 0
    for _ in range(20000):
        over_by_chunk = np.maximum(loads - 512.0, 0.0).sum(1)
        if over_by_chunk.sum() == 0:
            break
        A = int(np.argmax(over_by_chunk + 1e-6 * rng.random(NCHUNKS)))
        nodes_A = np.nonzero(chunk_of == A)[0]
        want = np.maximum(loads[A] - 512.0, 0.0)
        n_cand = nodes_A[np.argsort(-(degf[nodes_A] @ want))[:8]]
        fA0 = overflow(loads[A][None, :])
        best = (0.0, None, None)
        for n in n_cand:
            dn = degf[n]
            # candidate partners: every other node
            dm = degf  # [N, 3]
            newA = loads[A][None, :] - dn[None, :] + dm
            fB0 = np.maximum(loads[chunk_of] - 512.0, 0.0).sum(1)
            newB = loads[chunk_of] + dn[None, :] - dm
            delta = (np.maximum(newA - 512.0, 0.0).sum(1) - fA0
                     + np.maximum(newB - 512.0, 0.0).sum(1) - fB0)
            delta[chunk_of == A] = 1e18
            m = int(np.argmin(delta))
            if delta[m] < best[0] - 1e-9:
                best = (float(delta[m]), int(n), m)
        if best[1] is None:
            stall += 1
            if stall > 20:
                break
            continue
        stall = 0
        _, n, m = best
        B = chunk_of[m]
        loads[A] += degf[m] - degf[n]
        loads[B] += degf[n] - degf[m]
        chunk_of[n], chunk_of[m] = B, A

    # chunks ranked by worst slot load; rank j -> core j%8, q j//8
    rank = np.argsort(-loads.max(1), kind="stable")
    core_of_chunk = np.empty(NCHUNKS, np.int64)
    q_of_chunk = np.empty(NCHUNKS, np.int64)
    for j, ch in enumerate(rank):
        core_of_chunk[ch] = j % NCORES
        q_of_chunk[ch] = j // NCORES
    c_of = core_of_chunk[chunk_of]
    q_of = q_of_chunk[chunk_of]
    # r: position within chunk
    r_of = np.empty(N, np.int64)
    for ch in range(NCHUNKS):
        nodes = np.nonzero(chunk_of == ch)[0]
        r_of[nodes] = np.arange(len(nodes))
    return c_of, q_of, r_of


def _prep_inputs(nodes, bp_params, bp_bias, ho_params, ho_bias, edges,
                 edge_types):
    nodes = np.asarray(nodes, np.float32)
    bp_params = np.asarray(bp_params, np.float32)
    bp_bias = np.asarray(bp_bias, np.float32)
    ho_params = np.asarray(ho_params, np.float32)
    ho_bias = np.asarray(ho_bias, np.float32)
    edges = np.asarray(edges, np.int64)
    edge_types = np.asarray(edge_types, np.int64)

    nodes_b = nodes.astype(bf16)

    c_of, q_of, r_of = _balance(edges)

    # group sizes per (core, q, i)
    cnt = np.zeros((NCORES, 4, ORDER), np.int64)
    tgt_c = c_of[edges]   # [E, 3]
    tgt_q = q_of[edges]
    tgt_r = r_of[edges]
    for i in range(ORDER):
        np.add.at(cnt[:, :, i], (tgt_c[:, i], tgt_q[:, i]), 1)
    Tgs = [int(np.ceil(cnt[:, g // 3, g % 3].max() / 128)) for g in range(12)]
    pad = (-sum(Tgs)) % 4
    Tgs[11] += pad
    Tgs = tuple(Tgs)
    T = sum(Tgs)
    SB = T // 4
    L = 128 * T
    off = np.concatenate([[0], np.cumsum(Tgs)]) * 128  # col offset per group

    # weight tables (shared across cores)
    w1a = np.zeros((128, 128), np.float32)
    w1b = np.zeros((128, 128), np.float32)
    for p in range(NP_):
        w1a[13 * p:13 * p + 13, :] = bp_params[p]
        w1a[52 + p, :] = bp_bias[p, 0, :]
        w1b[64 + 13 * p:64 + 13 * p + 13, :] = bp_params[p]
        w1b[116 + p, :] = bp_bias[p, 0, :]
    w2 = np.zeros((128, 3, 64), np.float32)
    for i in range(ORDER):
        for p in range(NP_):
            w2[:, i, 16 * p:16 * p + 13] = ho_params[i, p]
    w1a_b = w1a.astype(bf16)
    w1b_b = w1b.astype(bf16)
    w2_b = w2.astype(bf16)

    in_maps = []
    for c in range(NCORES):
        xab = np.zeros((128, L), np.float32)
        msb = np.zeros((128, T, 5, 16), np.float32)
        oneh = np.zeros((128, T, 128), np.float32)
        for qq in range(4):
            for i in range(ORDER):
                g = qq * 3 + i
                es = np.nonzero((tgt_c[:, i] == c) & (tgt_q[:, i] == qq))[0]
                k = np.arange(len(es))
                x = off[g] + k
                t_arr = x // 128
                lane = x % 128
                a, b = (i + 1) % 3, (i + 2) % 3
                ta_t = edge_types[es, a]
                tb_t = edge_types[es, b]
                fa = nodes_b[edges[es, a]].astype(np.float32)  # [m, 13]
                fb = nodes_b[edges[es, b]].astype(np.float32)
                for dd in range(D):
                    xab[13 * ta_t + dd, x] = fa[:, dd]
                    xab[64 + 13 * tb_t + dd, x] = fb[:, dd]
                xab[52 + ta_t, x] = 1.0
                xab[116 + tb_t, x] = 1.0
                p_e = edge_types[es, i]
                for dd in range(D):
                    msb[lane, t_arr, p_e, dd] = 1.0
                    msb[lane, t_arr, 4, dd] = ho_bias[i, p_e, 0, dd]
                oneh[lane, t_arr, tgt_r[es, i]] = 1.0
        in_maps.append({
            "xab": xab.astype(bf16),
            "w1a": w1a_b, "w1b": w1b_b, "w2": w2_b,
            "msb": msb.reshape(128, SB, 4, 5, 16).astype(bf16),
            "oneh": oneh.reshape(128, SB, 4, 128).astype(bf16),
        })
    return in_maps, Tgs, (c_of, q_of, r_of)


def kernel(nodes, bp_params, bp_bias, ho_params, ho_bias, edges, edge_types,
           atoms=None, atom_edges=None, _run_kwargs=None):
    from concourse.bass_utils import run_bass_kernel_spmd

    in_maps, Tgs, (c_of, q_of, r_of) = _prep_inputs(
        nodes, bp_params, bp_bias, ho_params, ho_bias, edges, edge_types)
    if Tgs not in _COMPILED:
        _COMPILED[Tgs] = _build_program(Tgs)
    nc = _COMPILED[Tgs]

    res = run_bass_kernel_spmd(nc, in_maps, core_ids=list(range(NCORES)),
                               **(_run_kwargs or {}))
    outs = np.stack([res.results[c]["out"] for c in range(NCORES)])  # [8,128,64]
    full = outs[c_of, r_of][np.arange(N)[:, None],
                            16 * q_of[:, None] + np.arange(D)[None, :]]
    kernel._last_result = res
    return full.astype(np.float32)
